# revision 4
# baseline (speedup 1.0000x reference)
"""Trainium2 Bass kernel for nn_AttnDecoderRNN (S=2048, B=256, H=256, P=3).

Strategy: data-parallel over batch across 8 NeuronCores (32 rows each).
Device computes the memory/FLOP-heavy attention (energy matmul over
encoder_outputs, softmax, context); the tiny GRU/BatchNorm tail
(<0.5% of FLOPs, needs full-batch stats) runs on host in fp32.

Per-core layout: enc shard [2048, 32, 256] f32.  For each of the 32
batch rows, s is tiled as s = p*16 + t (p = SBUF partition, t = tile
column), giving fully contiguous 16 KiB/partition DMA reads.

Energy z = X @ We.T + hid uses TensorE `is_transpose` matmuls with the
enc tile as the (transpose-loaded) stationary operand so the fp32->bf16
cast-DMA'd natural-layout tile needs no explicit transpose.  Sigmoid on
ScalarE doubles as the PSUM->SBUF move; the Wv dot is a fused
multiply+reduce on VectorE; softmax skips max-subtraction (|energy|<~1,
exp is safe); context accumulates via PSUM matmuls with the softmax
column as stationary.
"""

import numpy as np
import ml_dtypes

S, B, H, PDIM = 2048, 256, 256, 3
NCORES = 8
BS = B // NCORES          # 32 batch rows per core
NT = 16                   # s tiles per batch row: s = p*16 + t
EPS = 1e-5

XBAR_T = False            # False: PE-transpose path; True: DMA-xbar transpose

_compiled = None


# ----------------------------------------------------------------------------
# Workaround: this toolchain's walrus rejects TileContext's tail drain (one
# Drain carrying every proc's sem wait -> "Too many sync wait commands").
# Split the waits: one sync-engine NOP per live proc, each with one wait.
# ----------------------------------------------------------------------------
def _install_tilefix():
    import re
    import bass_rust
    import concourse.tile as _tile
    from concourse.vector_clock import ScopedClock

    def _clock_values(vc):
        m = re.search(r"\[(.*)\]", str(vc))
        s = m.group(1).strip()
        return [int(x) for x in s.split(",")] if s else []

    def _split_drain_and_barrier(self, tick_clock, wait_clock):
        vals = _clock_values(tick_clock.global_clock)
        for i, v in enumerate(vals):
            if v > 0:
                part = [0] * len(vals)
                part[i] = v
                nop = self.nc.sync.nop(nofuse=True, hint=f"tail_wait_p{i}")
                wait_clock.add_sem_waits(
                    nop.ins, ScopedClock({None: bass_rust.VectorClock(part)})
                )
        self.nc.all_engine_barrier()
        assert self.sems is not None
        popped = self.nc._tile_sem_poison_stack.pop()
        assert popped is self._sem_poison
        self.nc.clear_and_free_semaphores(list(self.sems.allocated().values()))
        self.nc.all_engine_barrier()

    _tile.TileContext._drain_and_barrier = _split_drain_and_barrier


def _build_program():
    import concourse.bacc as bacc
    import concourse.tile as tile
    import concourse.mybir as mybir

    _install_tilefix()
    f32, bf16 = mybir.dt.float32, mybir.dt.bfloat16
    AF = mybir.ActivationFunctionType
    ALU = mybir.AluOpType

    nc = bacc.Bacc("TRN2", target_bir_lowering=False, debug=False,
                   num_devices=NCORES)
    enc = nc.declare_dram_parameter("enc", [S, BS, H], f32, isOutput=False)
    wet = nc.declare_dram_parameter("wet", [H, H], bf16, isOutput=False)     # We.T [h_in, feat]
    hidfT = nc.declare_dram_parameter("hidfT", [H, BS], f32, isOutput=False)  # (hid@Wh.T+bh+be).T
    wvcol = nc.declare_dram_parameter("wvcol", [H, 1], bf16, isOutput=False)
    ident = nc.declare_dram_parameter("ident", [128, 128], bf16, isOutput=False)
    attn_o = nc.declare_dram_parameter("attn", [BS, S], f32, isOutput=True)
    ctx_o = nc.declare_dram_parameter("ctx", [BS, H], f32, isOutput=True)

    with tile.TileContext(nc) as tc:
        import contextlib
        with contextlib.ExitStack() as ctx:
            consts = ctx.enter_context(tc.tile_pool(name="consts", bufs=1))
            xpool = ctx.enter_context(tc.tile_pool(name="x", bufs=2))
            tppool = ctx.enter_context(tc.tile_pool(name="tp", bufs=2, space="PSUM"))
            ztpool = ctx.enter_context(tc.tile_pool(name="zt", bufs=2, space="PSUM"))
            epspool = ctx.enter_context(tc.tile_pool(name="eps", bufs=1, space="PSUM"))
            zppool = ctx.enter_context(tc.tile_pool(name="zp", bufs=1, space="PSUM"))
            cpool = ctx.enter_context(tc.tile_pool(name="cp", bufs=1, space="PSUM"))
            spool = ctx.enter_context(tc.tile_pool(name="sg", bufs=4))
            jpool = ctx.enter_context(tc.tile_pool(name="junk", bufs=2))
            ppool = ctx.enter_context(tc.tile_pool(name="pall", bufs=2))
            smpool = ctx.enter_context(tc.tile_pool(name="small", bufs=8))

            # constants
            wet_sb = consts.tile([128, 2, H], bf16)          # [h_in_chunk p, k, feat]
            nc.sync.dma_start(out=wet_sb[:, 0, :], in_=wet[0:128, :])
            nc.sync.dma_start(out=wet_sb[:, 1, :], in_=wet[128:256, :])
            hidfT_sb = consts.tile([128, 2, BS], f32)        # [feat_chunk p, m, b]
            nc.sync.dma_start(out=hidfT_sb,
                              in_=hidfT.rearrange("(m p) b -> p m b", p=128))
            wv_sb = consts.tile([128, 2, 1], bf16)           # Wv chunks as columns
            nc.sync.dma_start(out=wv_sb,
                              in_=wvcol.rearrange("(m p) one -> p m one", p=128))
            id_sb = consts.tile([128, 128], bf16)
            nc.sync.dma_start(out=id_sb, in_=ident[:, :])
            onesf = consts.tile([128, 128], f32)
            nc.vector.memset(onesf, 1.0)

            for b in range(BS):
                # load + cast: s = p*16 + t  (16 KiB contiguous per partition)
                X = xpool.tile([128, NT, H], bf16)
                nc.gpsimd.dma_start(
                    out=X,
                    in_=enc[:, b, :].rearrange("(p t) h -> p t h", p=128),
                )
                pall = ppool.tile([128, NT], f32)
                for t in range(NT):
                    # transpose the two h-chunks of this tile: XT[:, k, :] = X[:, t, k*128:...].T
                    XT = spool.tile([128, 2, 128], bf16)
                    for k in range(2):
                        tp = tppool.tile([128, 128], bf16)
                        nc.tensor.transpose(tp, X[:, t, k * 128:(k + 1) * 128], id_sb)
                        nc.vector.tensor_copy(XT[:, k, :], tp)
                    e_ps = epspool.tile([128, 1], f32)
                    for m in range(2):   # output-feature chunk
                        zT = ztpool.tile([128, 128], f32)
                        nc.tensor.matmul(zT, wet_sb[:, 0, m * 128:(m + 1) * 128],
                                         XT[:, 0, :], start=True, stop=False)
                        nc.tensor.matmul(zT, wet_sb[:, 1, m * 128:(m + 1) * 128],
                                         XT[:, 1, :], start=False, stop=True)
                        sgT = jpool.tile([128, 2, 128], bf16)
                        nc.scalar.activation(sgT[:, m, :], zT, AF.Sigmoid,
                                             bias=hidfT_sb[:, m, b:b + 1])
                        nc.tensor.matmul(e_ps, sgT[:, m, :], wv_sb[:, m, :],
                                         start=(m == 0), stop=(m == 1))
                    # softmax numerator without max-subtraction (|energy| < ~1)
                    nc.scalar.activation(pall[:, t:t + 1], e_ps, AF.Exp)
                zp = zppool.tile([128, NT], f32)
                nc.tensor.matmul(zp, onesf, pall, start=True, stop=True)
                zr = smpool.tile([128, 1], f32)
                nc.vector.reduce_sum(zr, zp, mybir.AxisListType.X)
                zi = smpool.tile([128, 1], f32)
                nc.vector.reciprocal(zi, zr)
                attn_f = ppool.tile([128, NT], f32)
                nc.vector.tensor_scalar_mul(attn_f, pall, zi)
                nc.sync.dma_start(
                    out=attn_o[b, :].rearrange("(p t) -> p t", p=128), in_=attn_f
                )
                attn_bf = smpool.tile([128, NT], bf16)
                nc.vector.tensor_copy(attn_bf, attn_f)
                ctxp = cpool.tile([1, H], f32)
                for t in range(NT):
                    nc.tensor.matmul(ctxp, attn_bf[:, t:t + 1], X[:, t, :],
                                     start=(t == 0), stop=(t == NT - 1))
                ctmp = smpool.tile([1, H], f32)
                nc.scalar.copy(ctmp, ctxp)
                nc.sync.dma_start(out=ctx_o[b:b + 1, :], in_=ctmp)
    nc.compile()
    return nc


def _host_tail(last_palette, context, last_decoder_hidden,
               W_ih, b_ih, W_hh, b_hh, W1, b1, gamma, beta, W2, b2):
    x = np.concatenate([last_palette, context], axis=1).astype(np.float32)
    gi = x @ W_ih.T + b_ih
    gh = last_decoder_hidden @ W_hh.T + b_hh
    ir, iz, in_ = np.split(gi, 3, axis=1)
    hr, hz, hn = np.split(gh, 3, axis=1)
    r = 1.0 / (1.0 + np.exp(-(ir + hr)))
    zg = 1.0 / (1.0 + np.exp(-(iz + hz)))
    n = np.tanh(in_ + r * hn)
    gru_hidden = (1.0 - zg) * n + zg * last_decoder_hidden
    h1 = np.maximum(gru_hidden @ W1.T + b1, 0.0)
    mu = h1.mean(axis=0)
    var = h1.var(axis=0)
    h1n = (h1 - mu) / np.sqrt(var + EPS) * gamma + beta
    palette = h1n @ W2.T + b2
    return palette.astype(np.float32), gru_hidden.astype(np.float32)


def kernel(last_palette, last_decoder_hidden, encoder_outputs,
           We, be, Wh, bh, Wv, bv,
           W_ih, b_ih, W_hh, b_hh,
           W1, b1, gamma, beta, W2, b2,
           each_input_size, i, **_unused):
    global _compiled
    from concourse.bass_utils import run_bass_kernel_spmd

    lp = np.asarray(last_palette, np.float32)
    hid = np.asarray(last_decoder_hidden, np.float32)
    enc = np.ascontiguousarray(np.asarray(encoder_outputs, np.float32))
    We, be, Wh, bh = (np.asarray(a, np.float32) for a in (We, be, Wh, bh))
    Wv, bv = np.asarray(Wv, np.float32), np.asarray(bv, np.float32)

    if _compiled is None:
        _compiled = _build_program()
    nc = _compiled

    wet_np = np.ascontiguousarray(We.T).astype(ml_dtypes.bfloat16)
    hidfT_np = np.ascontiguousarray((hid @ Wh.T + bh + be).T).astype(np.float32)  # [H, B]
    wvcol_np = np.ascontiguousarray(Wv.reshape(H, 1)).astype(ml_dtypes.bfloat16)
    ident_np = np.eye(128, dtype=np.float32).astype(ml_dtypes.bfloat16)

    in_maps = []
    for c in range(NCORES):
        sl = slice(c * BS, (c + 1) * BS)
        in_maps.append({
            "enc": np.ascontiguousarray(enc[:, sl, :]),
            "wet": wet_np,
            "hidfT": np.ascontiguousarray(hidfT_np[:, sl]),
            "wvcol": wvcol_np,
            "ident": ident_np,
        })
    res = run_bass_kernel_spmd(nc, in_maps, list(range(NCORES))).results

    attn = np.concatenate([r["attn"] for r in res], axis=0)          # [B, S]
    context = np.concatenate([r["ctx"] for r in res], axis=0)        # [B, H]

    palette, gru_hidden = _host_tail(
        lp, context, hid,
        *(np.asarray(a, np.float32) for a in (W_ih, b_ih, W_hh, b_hh,
                                              W1, b1, gamma, beta, W2, b2)))

    attn_weights = attn[:, None, :]                                   # [B,1,S]
    context_out = context[None, :, None, :]                           # [1,B,1,H]
    return palette, context_out, gru_hidden, attn_weights
